# revision 1
# baseline (speedup 1.0000x reference)
"""DispMVS depth-fusion kernel for 8 Trainium2 NeuronCores.

Sharding: core c handles batch b = c // 4 and coarse rows r0 = (c % 4) * 64
(64 of 256 rows), with BOTH neighbor streams (NN=2) resident on the core
(partitions = nn*64 + row).  The cross-neighbor confidence-fusion softmax is
then core-local; cores never communicate.

Pipeline per core (one Bass/Tile program, identical for all 8 cores):
  1. geometry: elementwise epipolar math -> clipped inverse depth  [128, 330]
     (330 = 320 cols + 10 packed halo pixels/partition for the rows just
     outside the chunk, needed by the 3x3 unfold).
  2. DRAM scratch round-trip to rebuild inv-depth as 3 vertically shifted,
     zero-padded rows per partition (the unfold operand); conf comes the same
     way directly from a host-padded input.
  3. per (p, w-half) chunk: exp(mask) on ACT, grouped 9-way reductions on DVE
     (softmax numerators/denominator), convex-upsample of inv-depth and conf,
     then the 2-view softmax fusion and final reciprocal.
"""

import numpy as np

NN, B, H, W = 2, 2, 256, 320
UP = 4
EPS = 1e-6
RPC = 64          # coarse rows per core
NCORES = 8
HW = H * W
RW = RPC * W      # elements in one [64, 320] channel-slice

# consts columns
(
    C_M00, C_M01, C_M02, C_M10, C_M11, C_M12, C_M20, C_M21, C_M22,
    C_T0, C_T1, C_T2,
    C_R00, C_R01, C_R02, C_R10, C_R11, C_R12, C_R20, C_R21, C_R22,
    C_A0, C_A1, C_A2, C_B0, C_B1, C_B2,
    C_TX, C_TY, C_TZ,
    C_CA, C_CB, C_DS, C_DB, C_TEN,
) = range(35)
NCONST = 36

_cache = {}


def _register_custom_ops():
    """Register this kernel's custom DVE ops (idempotent). Returns a dict.

    MUL_CUMSUM_ANT: out = cumsum(in0*in1) along the free stream - grouped
      9-tap weighted sums fall out as differences of every-9th prefix value,
      one line-rate pass instead of multiply + strided TENSOR_REDUCE passes.
    SUMSQ_ANT: out = in0^2 + in1^2 (one pass instead of 3).
    RSQRT_NR_ANT: one Newton step for 1/sqrt: out = in0*(1.5 - 0.5*in1*in0^2)
      (one pass instead of 4).
    """
    from concourse import dve_ops
    from concourse.dve_spec import AluOp, C0, C1, Spec, Src0, Src1, _has_src1, lower, scan
    from concourse.dve_uop import DveOpSpec

    have = {o.name: o for o in dve_ops.OPS}
    if "MUL_CUMSUM_ANT" in have:
        return have

    def cum_ref(in0, in1, s0, s1, imm2):
        a = in0.astype(np.float32).reshape(in0.shape[0], -1) * in1.astype(
            np.float32
        ).reshape(in1.shape[0], -1)
        return np.cumsum(a, axis=1, dtype=np.float32).reshape(in0.shape)

    specs = [
        ("MUL_CUMSUM_ANT", Spec(body=scan(AluOp.ADD, Src0 * Src1), reference=cum_ref)),
        (
            "SUMSQ_ANT",
            Spec(
                body=Src0 * Src0 + Src1 * Src1,
                reference=lambda in0, in1, s0, s1, imm2: (
                    in0.astype(np.float32) ** 2 + in1.astype(np.float32) ** 2
                ),
            ),
        ),
        (
            "RSQRT_NR_ANT",
            Spec(
                body=(Src0 * Src0 * Src1 * C0 + C1) * Src0,
                reference=lambda in0, in1, s0, s1, imm2: (
                    (in0.astype(np.float32) ** 2 * in1 * s0 + s1) * in0
                ),
            ),
        ),
    ]
    out = {}
    for name, spec in specs:
        op = dve_ops.DveOp(name, spec, subdim=False, uops_sha={})
        dve_ops.OPS.append(op)
        dve_ops.CUSTOM_DVE_SPECS[name] = spec
        dve_ops._SUB_OPCODE_FOR_NAME[name] = (
            dve_ops._CUSTOM_DVE_ROW_BASE + len(dve_ops.OPS) - 1
        )
        for ver in ("v3", "v4"):
            tmp = DveOpSpec(
                name=name,
                opcode=dve_ops.get_dve_sub_opcode(name),
                uops=lower(spec, ver=ver),
                rd1_en=_has_src1(spec),
            )
            op.uops_sha[ver] = tmp.sha(ver)
        out[name] = op
    assert max(dve_ops._SUB_OPCODE_FOR_NAME.values()) < 0x20
    return out


def _build_program():
    import concourse.bass as bass
    import concourse.bacc as bacc
    import concourse.tile as tile
    from concourse import mybir
    from concourse.alu_op_type import AluOpType as op

    f32 = mybir.dt.float32
    i32 = mybir.dt.int32
    Act = mybir.ActivationFunctionType

    cops = _register_custom_ops()
    nc = bacc.Bacc("TRN2", target_bir_lowering=False, debug=False)

    pix_d = nc.dram_tensor("pix", [128, 4, 330], f32, kind="ExternalInput").ap()
    consts_d = nc.dram_tensor("consts", [128, NCONST], f32, kind="ExternalInput").ap()
    hm_d = nc.dram_tensor("hm", [128, 10], f32, kind="ExternalInput").ap()
    confpad_d = nc.dram_tensor("confpad", [NN, 66, 322], f32, kind="ExternalInput").ap()
    # mask pre-packed on host to [gc, wc, (nn,r), q, w, k] (k innermost) so each
    # chunk's DMA is one contiguous [128, 5760] transfer and the 9-tap groups
    # are unit-stride for the cumsum trick
    mask_d = nc.dram_tensor("maskpk", [4, 2, 128, 4, 160, 9], f32, kind="ExternalInput").ap()
    scr = nc.dram_tensor("scr", [NN, 66, 322], f32, kind="Internal").ap()
    out_d = nc.dram_tensor("out", [RPC * UP, W * UP], f32, kind="ExternalOutput").ap()

    def dram_ap(base, off, dims):
        return bass.AP(tensor=base.tensor, offset=base.offset + off, ap=[list(d) for d in dims])

    with tile.TileContext(nc) as tc:
        with tc.tile_pool(name="persist", bufs=1) as pp:
            ep_ctx = tc.tile_pool(name="early", bufs=1)
            ep = ep_ctx.__enter__()
            consts = pp.tile([128, NCONST], f32, name="consts")
            nc.sync.dma_start(out=consts[:], in_=consts_d)

            def CC(i, p0=0, p1=128):
                return consts[p0:p1, i : i + 1]

            pix = ep.tile([128, 4, 330], f32, name="pix")
            nc.sync.dma_start(out=pix[:], in_=pix_d)
            hm = ep.tile([128, 10], f32, name="hm")
            nc.sync.dma_start(out=hm[:], in_=hm_d)

            t3i = ep.tile([128, 3, 322], f32, name="t3i")  # unfold rows of inv-depth
            t3c = ep.tile([128, 3, 322], f32, name="t3c")  # unfold rows of conf
            # conf unfold rows straight from the host-padded input
            for nn in range(NN):
                src = dram_ap(
                    confpad_d, nn * 66 * 322,
                    [[322, 64], [322, 3], [1, 322]],
                )
                nc.sync.dma_start(out=t3c[nn * 64 : nn * 64 + 64], in_=src)

            inv_res = ep.tile([128, 330], f32, name="inv_res")
            zero2 = ep.tile([2, 132], f32, name="zero2")
            nc.vector.memset(zero2[:], 0.0)

            # ---------------- geometry ----------------
            u = pix[:, 0, :]
            v = pix[:, 1, :]
            d = pix[:, 2, :]
            fl = pix[:, 3, :]

            with tc.tile_pool(name="geom", bufs=1) as gp:
                _tagn = [0]

                def T(shape=(128, 330)):
                    _tagn[0] += 1
                    return gp.tile(list(shape), f32, name=f"g{_tagn[0]}", tag=f"g{_tagn[0]}")

                def TT(o, a, b, alu):
                    nc.vector.tensor_tensor(out=o, in0=a, in1=b, op=alu)

                def TS(o, a, s1, o0, s2=None, o1=None):
                    if o1 is None:
                        nc.vector.tensor_scalar(out=o, in0=a, scalar1=s1, scalar2=None, op0=o0)
                    else:
                        nc.vector.tensor_scalar(out=o, in0=a, scalar1=s1, scalar2=s2, op0=o0, op1=o1)

                def STT(o, a, s, b, o0, o1):
                    nc.vector.scalar_tensor_tensor(out=o, in0=a, scalar=s, in1=b, op0=o0, op1=o1)

                def AB(o, a):
                    nc.scalar.activation(out=o, in_=a, func=Act.Abs)

                def AF(o, a, scale, bias):
                    nc.scalar.activation(out=o, in_=a, func=Act.Identity, scale=scale, bias=bias)

                def recip_acc(o, x):
                    t = T()
                    nc.vector.reciprocal_approx_accurate(out=o, in_=x, scratch=t[:])

                # a_j = M @ [u, v, 1]
                a0, a1, a2 = T(), T(), T()
                tmp = T()
                AF(tmp[:], u, CC(C_M00), CC(C_M02))
                STT(a0[:], v, CC(C_M01), tmp[:], op.mult, op.add)
                AF(tmp[:], u, CC(C_M10), CC(C_M12))
                STT(a1[:], v, CC(C_M11), tmp[:], op.mult, op.add)
                AF(tmp[:], u, CC(C_M20), CC(C_M22))
                STT(a2[:], v, CC(C_M21), tmp[:], op.mult, op.add)

                d10 = T()
                AF(d10[:], d, 1.0, CC(C_TEN))

                # z components and their reciprocals
                ps2, pe2, rs2, re2 = T(), T(), T(), T()
                m = T()
                TT(m[:], a2[:], d, op.mult)
                AF(ps2[:], m[:], 1.0, CC(C_T2))
                TT(m[:], a2[:], d10[:], op.mult)
                TT(pe2[:], m[:], ps2[:], op.add)
                AB(m[:], ps2[:])
                TS(m[:], m[:], EPS, op.add)
                recip_acc(rs2[:], m[:])
                AB(m[:], pe2[:])
                TS(m[:], m[:], EPS, op.add)
                recip_acc(re2[:], m[:])

                # x/y components, start and end projections
                pxs, pys, pxe, pye = T(), T(), T(), T()
                for aj, tj, po_s, po_e in ((a0, C_T0, pxs, pxe), (a1, C_T1, pys, pye)):
                    psj, pej = T((128, 330)), T((128, 330))
                    TT(m[:], aj[:], d, op.mult)
                    AF(psj[:], m[:], 1.0, CC(tj))
                    TT(m[:], aj[:], d10[:], op.mult)
                    TT(pej[:], m[:], psj[:], op.add)
                    TT(po_s[:], psj[:], rs2[:], op.mult)
                    TT(po_e[:], pej[:], re2[:], op.mult)

                fdx, fdy = T(), T()
                TT(fdx[:], pxe[:], pxs[:], op.subtract)
                TT(fdy[:], pye[:], pys[:], op.subtract)

                # rsqrt(fdx^2 + fdy^2) via magic seed + 2 fused Newton steps
                q = T()
                nc.vector._custom_dve(cops["SUMSQ_ANT"], out=q[:], in0=fdx[:], in1=fdy[:])
                y = T()
                yi = y[:].bitcast(i32)
                TS(yi, q[:].bitcast(i32), 1, op.arith_shift_right)
                TS(yi, yi, -1, op.bitwise_xor)
                TS(yi, yi, 0x5F3759DF + 1, op.add)
                y2 = T()
                nc.vector._custom_dve(
                    cops["RSQRT_NR_ANT"], out=y2[:], in0=y[:], in1=q[:], s0=-0.5, s1=1.5
                )
                nc.vector._custom_dve(
                    cops["RSQRT_NR_ANT"], out=y[:], in0=y2[:], in1=q[:], s0=-0.5, s1=1.5
                )

                fls = T()
                TT(fls[:], fl, y[:], op.mult)
                mx, my = T(), T()
                TT(m[:], fdx[:], fls[:], op.mult)
                TT(mx[:], m[:], pxs[:], op.add)
                TT(m[:], fdy[:], fls[:], op.mult)
                TT(my[:], m[:], pys[:], op.add)

                fm = T()
                fmi = fm[:].bitcast(i32)
                ax = T()
                AB(ax[:], fdx[:])
                AB(m[:], fdy[:])
                TT(fmi, m[:], ax[:], op.is_gt)

                nx, ny = T(), T()
                AF(tmp[:], mx[:], CC(C_A0), CC(C_A2))
                STT(nx[:], my[:], CC(C_A1), tmp[:], op.mult, op.add)
                AF(tmp[:], mx[:], CC(C_B0), CC(C_B2))
                STT(ny[:], my[:], CC(C_B1), tmp[:], op.mult, op.add)

                rx, ry, rz = T(), T(), T()
                AF(tmp[:], u, CC(C_R00), CC(C_R02))
                STT(rx[:], v, CC(C_R01), tmp[:], op.mult, op.add)
                AF(tmp[:], u, CC(C_R10), CC(C_R12))
                STT(ry[:], v, CC(C_R11), tmp[:], op.mult, op.add)
                AF(tmp[:], u, CC(C_R20), CC(C_R22))
                STT(rz[:], v, CC(C_R21), tmp[:], op.mult, op.add)

                def inv_axis(o, nj, rj, c_t):
                    num = T()
                    TT(m[:], rz[:], nj[:], op.mult)
                    TT(m[:], rj[:], m[:], op.subtract)
                    AB(num[:], m[:])
                    AF(m[:], nj[:], CC(C_TZ), CC(c_t))
                    AB(m[:], m[:])
                    TS(m[:], m[:], EPS, op.add)
                    rden = T()
                    recip_acc(rden[:], m[:])
                    TT(o, num[:], rden[:], op.mult)

                invx, invy = T(), T()
                inv_axis(invx[:], nx, rx, C_TX)
                inv_axis(invy[:], ny, ry, C_TY)

                seld = T()
                nc.vector.select(out=seld[:], mask=fmi, on_true=invy[:], on_false=invx[:])
                AF(tmp[:], seld[:], CC(C_CA), CC(C_CB))
                TS(inv_res[:], tmp[:], 0.0, op.max, 1.0, op.min)

            # zero the halo pixels that fall outside the image (edge chunks)
            nc.vector.tensor_tensor(
                out=inv_res[:, 320:330], in0=inv_res[:, 320:330], in1=hm[:], op=op.mult
            )

            # ------- scratch round-trip: [nn, 66, 322] padded inv-depth -------
            for nn in range(NN):
                base = nn * 66 * 322
                sl = slice(nn * 64, nn * 64 + 64)
                nc.sync.dma_start(
                    out=dram_ap(scr, base + 322 + 1, [[322, 64], [1, 320]]),
                    in_=inv_res[sl, 0:320],
                )
                nc.sync.dma_start(
                    out=dram_ap(scr, base + 1, [[5, 64], [1, 5]]),
                    in_=inv_res[sl, 320:325],
                )
                nc.sync.dma_start(
                    out=dram_ap(scr, base + 65 * 322 + 1, [[5, 64], [1, 5]]),
                    in_=inv_res[sl, 325:330],
                )
                # zero pad columns 0 and 321 of all 66 rows
                nc.sync.dma_start(
                    out=dram_ap(scr, base, [[0, 1], [322, 66], [321, 2]]),
                    in_=zero2[nn : nn + 1, :].rearrange("p (a b) -> p a b", a=66),
                )
            for nn in range(NN):
                src = dram_ap(scr, nn * 66 * 322, [[322, 64], [322, 3], [1, 322]])
                nc.sync.dma_start(out=t3i[nn * 64 : nn * 64 + 64], in_=src)

            # unfold weights interleaved [w, k] (k innermost) so the
            # weighted-cumsum's src1 for any w-window is one contiguous slice
            ufi9i = pp.tile([128, 322, 9], f32, name="ufi9i")
            ufi9c = pp.tile([128, 322, 9], f32, name="ufi9c")
            for t3, ufi9 in ((t3i, ufi9i), (t3c, ufi9c)):
                for dy in range(3):
                    for dx in range(3):
                        nc.scalar.activation(
                            out=ufi9[:, 0 : 322 - dx, dy * 3 + dx],
                            in_=t3[:, dy, dx:322],
                            func=Act.Copy,
                        )

            ep_ctx.__exit__(None, None, None)

            # ---------------- upsample + fusion, 2 w-halves x 4 p-chunks ----------------
            WC = 160
            with tc.tile_pool(name="chunk", bufs=2) as cp, tc.tile_pool(
                name="chunk1", bufs=1
            ) as cp1:
                for wc in range(2):
                    w0 = wc * WC
                    ufs = {"i": ufi9i[:, w0 : w0 + WC, :], "c": ufi9c[:, w0 : w0 + WC, :]}
                    for gc in range(4):
                        e = cp.tile([128, 4, WC, 9], f32, name="e", tag="e")
                        nc.sync.dma_start(out=e[:], in_=mask_d[gc, wc])
                        nc.scalar.activation(out=e[:], in_=e[:], func=Act.Exp)

                        # softmax denominator: unit-stride innermost-k reduce
                        s = cp.tile([128, 4, WC], f32, name="s", tag="s")
                        nc.vector.tensor_reduce(
                            out=s[:], in_=e[:], axis=mybir.AxisListType.X, op=op.add
                        )
                        rs = cp.tile([128, 4, WC], f32, name="rs", tag="rs")
                        nc.vector.reciprocal_approx_fast(out=rs[:], in_=s[:])

                        up_t = {}
                        for tag in ("i", "c"):
                            cum = cp1.tile(
                                [128, 4, WC, 9], f32, name="cum", tag="cum", bufs=2
                            )
                            for g in range(4):
                                nc.vector._custom_dve(
                                    cops["MUL_CUMSUM_ANT"], out=cum[:, g], in0=e[:, g], in1=ufs[tag]
                                )
                            # every-9th prefix value, with a zero column prepended
                            ce = cp1.tile([128, 4, WC + 1], f32, name="ce", tag="ce" + tag)
                            nc.vector.memset(ce[:, :, 0:1], 0.0)
                            nc.scalar.activation(
                                out=ce[:, :, 1 : WC + 1], in_=cum[:, :, :, 8], func=Act.Copy
                            )
                            acc = cp.tile([128, 4, WC], f32, name="acc", tag="acc" + tag)
                            nc.vector.tensor_tensor(
                                out=acc[:],
                                in0=ce[:, :, 1 : WC + 1],
                                in1=ce[:, :, 0:WC],
                                op=op.subtract,
                            )
                            upv = cp.tile([128, 4, WC], f32, name="upv", tag="up" + tag)
                            nc.vector.tensor_tensor(out=upv[:], in0=acc[:], in1=rs[:], op=op.mult)
                            up_t[tag] = upv

                        iu, cu = up_t["i"], up_t["c"]
                        lo, hi = slice(0, 64), slice(64, 128)

                        def F(tag):
                            return cp.tile([64, 4, WC], f32, name="f" + tag, tag="f" + tag)

                        # TT operands must share a base partition: move the nn1
                        # halves down to partitions 0-63 via SBUF->SBUF DMA
                        iu2, cu2 = F("iu2"), F("cu2")
                        nc.sync.dma_start(out=iu2[:], in_=iu[hi])
                        nc.sync.dma_start(out=cu2[:], in_=cu[hi])

                        fa, fb, fc, fd = F("a"), F("b"), F("c"), F("d")
                        # fa=dif -> fb=exp(dif) -> fc=1+fb -> fd=1/fc
                        nc.vector.tensor_tensor(out=fa[:], in0=cu2[:], in1=cu[lo], op=op.subtract)
                        nc.scalar.activation(out=fb[:], in_=fa[:], func=Act.Exp)
                        nc.scalar.activation(out=fc[:], in_=fb[:], func=Act.Identity, bias=1.0)
                        nc.vector.reciprocal_approx_fast(out=fd[:], in_=fc[:])
                        # fa=iu1*e -> fc=fa+iu0 -> fa=fc*fd -> fc=scale*fa+bias
                        nc.vector.tensor_tensor(out=fa[:], in0=iu2[:], in1=fb[:], op=op.mult)
                        nc.vector.tensor_tensor(out=fc[:], in0=fa[:], in1=iu[lo], op=op.add)
                        nc.vector.tensor_tensor(out=fa[:], in0=fc[:], in1=fd[:], op=op.mult)
                        nc.scalar.activation(
                            out=fc[:], in_=fa[:], func=Act.Identity,
                            scale=CC(C_DS, 0, 64), bias=CC(C_DB, 0, 64),
                        )
                        out_t = cp.tile([64, WC, 4], f32, name="out_t", tag="out_t")
                        nc.vector.reciprocal_approx_fast(
                            out=out_t[:].rearrange("p w q -> p q w"), in_=fc[:]
                        )
                        dst = dram_ap(
                            out_d,
                            gc * (W * UP) + UP * w0,
                            [[UP * W * UP, 64], [UP, WC], [1, UP]],
                        )
                        nc.sync.dma_start(out=dst, in_=out_t[:])

    nc.finalize()
    return nc


def _host_prep(inputs):
    K_ref = np.asarray(inputs["K_ref"], np.float32)
    K_nei = np.asarray(inputs["K_nei"], np.float32)
    R_nei = np.asarray(inputs["R_nei"], np.float32)
    T_nei = np.asarray(inputs["T_nei"], np.float32)
    depth0 = np.asarray(inputs["depth0"], np.float32)
    flow = np.asarray(inputs["flow"], np.float32)
    mask = np.asarray(inputs["mask"], np.float32)
    conf = np.asarray(inputs["conf"], np.float32)
    dmin = float(np.asarray(inputs["depth_min"]).reshape(-1)[0])
    dmax = float(np.asarray(inputs["depth_max"]).reshape(-1)[0])

    # pixel rays per batch (u, v with unit z)
    uv = []
    for b in range(B):
        Ki = np.linalg.inv(K_ref[b, 0, 0].astype(np.float64))
        gx, gy = np.meshgrid(np.arange(W, dtype=np.float64), np.arange(H, dtype=np.float64))
        x = Ki[0, 0] * gx + Ki[0, 1] * gy + Ki[0, 2]
        y = Ki[1, 0] * gx + Ki[1, 1] * gy + Ki[1, 2]
        z = Ki[2, 0] * gx + Ki[2, 1] * gy + Ki[2, 2]
        uv.append((np.float32(x / z), np.float32(y / z)))

    cA = 1.0 / (dmin - dmax)
    cB = -dmax / (dmin - dmax)

    in_maps = []
    for c in range(NCORES):
        b, rc = c // 4, c % 4
        r0 = rc * RPC
        rtop = max(r0 - 1, 0)
        rbot = min(r0 + RPC, H - 1)

        consts = np.zeros((128, NCONST), np.float32)
        for nn in range(NN):
            Kn = K_nei[nn, b, 0, 0].astype(np.float64)
            Rn = R_nei[nn, b, 0, 0].astype(np.float64)
            Tn = T_nei[nn, b, 0, 0].astype(np.float64).reshape(3)
            M = Kn @ Rn
            t = (Kn @ Tn.reshape(3, 1)).reshape(3)
            iK = np.linalg.inv(Kn)
            assert abs(iK[2, 0]) < 1e-12 and abs(iK[2, 1]) < 1e-12 and abs(iK[2, 2] - 1) < 1e-9
            row = np.zeros(NCONST, np.float32)
            row[C_M00:C_M22 + 1] = M.reshape(-1)
            row[C_T0:C_T2 + 1] = t
            row[C_R00:C_R22 + 1] = Rn.reshape(-1)
            row[C_A0:C_A2 + 1] = iK[0] / (1.0 + EPS)
            row[C_B0:C_B2 + 1] = iK[1] / (1.0 + EPS)
            # C_TX/C_TY feed |tz*n + c| as ACT affine bias -> store negated
            row[C_TX], row[C_TY], row[C_TZ] = -Tn[0], -Tn[1], Tn[2]
            row[C_CA], row[C_CB] = cA, cB
            row[C_TEN] = 10.0
            row[C_DS], row[C_DB] = dmin - dmax, dmax
            consts[nn * 64 : nn * 64 + 64] = row

        u_full, v_full = uv[b]
        d_full = depth0[b, 0]

        pix = np.zeros((128, 4, 330), np.float32)
        for nn in range(NN):
            sl = slice(nn * 64, nn * 64 + 64)
            f_full = flow[nn, b, 0]
            for ch, arr in enumerate((u_full, v_full, d_full, f_full)):
                pix[sl, ch, 0:320] = arr[r0 : r0 + RPC]
                pix[sl, ch, 320:325] = arr[rtop].reshape(64, 5)
                pix[sl, ch, 325:330] = arr[rbot].reshape(64, 5)

        hm = np.ones((128, 10), np.float32)
        if r0 == 0:
            hm[:, 0:5] = 0.0
        if r0 + RPC == H:
            hm[:, 5:10] = 0.0

        confpad = np.zeros((NN, 66, 322), np.float32)
        confpad[:, 1:65, 1:321] = conf[:, b, 0, r0 : r0 + RPC, :]
        if r0 > 0:
            confpad[:, 0, 1:321] = conf[:, b, 0, r0 - 1, :]
        if r0 + RPC < H:
            confpad[:, 65, 1:321] = conf[:, b, 0, r0 + RPC, :]

        # [nn, k, p, q, r, wc, w] -> [p, wc, nn, r, q, w, k]
        ms = mask[:, b, :, r0 : r0 + RPC, :].reshape(NN, 9, 4, 4, RPC, 2, 160)
        mask_pk = np.ascontiguousarray(ms.transpose(2, 5, 0, 4, 3, 6, 1)).reshape(
            4, 2, 128, 4, 160, 9
        )

        in_maps.append(
            {
                "pix": pix,
                "consts": consts,
                "hm": hm,
                "confpad": confpad,
                "maskpk": mask_pk,
            }
        )
    return in_maps


def kernel(**inputs):
    if "nc" not in _cache:
        _cache["nc"] = _build_program()
    nc = _cache["nc"]
    in_maps = _host_prep(inputs)

    from concourse import bass_utils

    res = bass_utils.run_bass_kernel_spmd(nc, in_maps, core_ids=list(range(NCORES)))
    out = np.empty((B, 1, H * UP, W * UP), np.float32)
    for c in range(NCORES):
        b, rc = c // 4, c % 4
        out[b, 0, rc * RPC * UP : (rc + 1) * RPC * UP, :] = res.results[c]["out"]
    return out



# revision 6
# speedup vs baseline: 1.3756x; 1.3756x over previous
"""DispMVS depth-fusion kernel for 8 Trainium2 NeuronCores.

Sharding: core c handles batch b = c // 4 and coarse rows r0 = (c % 4) * 64
(64 of 256 rows), with BOTH neighbor streams (NN=2) resident on the core
(partitions = nn*64 + row).  The cross-neighbor confidence-fusion softmax is
then core-local; cores never communicate.

Pipeline per core (one Bass/Tile program, identical for all 8 cores):
  1. geometry: elementwise epipolar math -> clipped inverse depth (fp32 math,
     fp16 result), ops spread across DVE/GpSimd/ACT for ILP.  [128, 330]
  2. on-chip halo build: SBUF->SBUF DMAs create 3 vertically shifted rows of
     inv-depth (t3i); conf rows come host-padded (t3c).  9 shifted copies of
     each build the per-tap operand planes ufi/ufc [128, 9, 320] fp16.
  3. chunk loop (4 row-subpixel groups x 2 w-halves), all mask-sized work in
     fp16 with k-major planes [128, 9k, 4q, 160w]:
       ACT    exp(mask)
       DVE    e*ufi, e*ufc (fp16 TT at 2x), joint tree level-1
       GpSimd tree levels 2-4 -> (s, num_i, num_c), weighted avgs, fusion TTs
       ACT    tanh-based 2-view softmax weight, output affine
       DVE    reciprocals (softmax denom, final 1/depth)
     Tails (final recip + output DMA) are deferred one chunk to keep engine
     queues from stalling on cross-engine deps.
"""

import numpy as np

NN, B, H, W = 2, 2, 256, 320
UP = 4
EPS = 1e-6
RPC = 64          # coarse rows per core
NCORES = 8
WC = 160          # w-half width
KQ = 9 * 4 * WC   # elements per chunk per partition (5760)

# consts columns
(
    C_M00, C_M01, C_M02, C_M10, C_M11, C_M12, C_M20, C_M21, C_M22,
    C_T0, C_T1, C_T2,
    C_R00, C_R01, C_R02, C_R10, C_R11, C_R12, C_R20, C_R21, C_R22,
    C_A0, C_A1, C_A2, C_B0, C_B1, C_B2,
    C_TX, C_TY, C_TZ,
    C_CA, C_CB, C_DS2, C_DB, C_TEN,
) = range(35)
NCONST = 36

_cache = {}


def _register_custom_ops():
    """Register this kernel's custom DVE ops (idempotent). Returns a dict.

    SUMSQ_ANT: out = in0^2 + in1^2 (one pass instead of 3).
    RSQRT_NR_ANT: one Newton step for 1/sqrt: out = in0*(1.5 - 0.5*in1*in0^2)
      (one pass instead of 4).
    MUL_CUMSUM_ANT kept for sub-opcode stability with earlier builds.
    """
    from concourse import dve_ops
    from concourse.dve_spec import AluOp, C0, C1, Spec, Src0, Src1, _has_src1, lower, scan
    from concourse.dve_uop import DveOpSpec

    have = {o.name: o for o in dve_ops.OPS}
    if "MUL_CUMSUM_ANT" in have:
        return have

    def cum_ref(in0, in1, s0, s1, imm2):
        a = in0.astype(np.float32).reshape(in0.shape[0], -1) * in1.astype(
            np.float32
        ).reshape(in1.shape[0], -1)
        return np.cumsum(a, axis=1, dtype=np.float32).reshape(in0.shape)

    specs = [
        ("MUL_CUMSUM_ANT", Spec(body=scan(AluOp.ADD, Src0 * Src1), reference=cum_ref)),
        (
            "SUMSQ_ANT",
            Spec(
                body=Src0 * Src0 + Src1 * Src1,
                reference=lambda in0, in1, s0, s1, imm2: (
                    in0.astype(np.float32) ** 2 + in1.astype(np.float32) ** 2
                ),
            ),
        ),
        (
            "RSQRT_NR_ANT",
            Spec(
                body=(Src0 * Src0 * Src1 * C0 + C1) * Src0,
                reference=lambda in0, in1, s0, s1, imm2: (
                    (in0.astype(np.float32) ** 2 * in1 * s0 + s1) * in0
                ),
            ),
        ),
    ]
    out = {}
    for name, spec in specs:
        op_ = dve_ops.DveOp(name, spec, subdim=False, uops_sha={})
        dve_ops.OPS.append(op_)
        dve_ops.CUSTOM_DVE_SPECS[name] = spec
        dve_ops._SUB_OPCODE_FOR_NAME[name] = (
            dve_ops._CUSTOM_DVE_ROW_BASE + len(dve_ops.OPS) - 1
        )
        for ver in ("v3", "v4"):
            tmp = DveOpSpec(
                name=name,
                opcode=dve_ops.get_dve_sub_opcode(name),
                uops=lower(spec, ver=ver),
                rd1_en=_has_src1(spec),
            )
            op_.uops_sha[ver] = tmp.sha(ver)
        out[name] = op_
    assert max(dve_ops._SUB_OPCODE_FOR_NAME.values()) < 0x20
    return out


def _build_program():
    import concourse.bass as bass
    import concourse.bacc as bacc
    import concourse.tile as tile
    from concourse import mybir
    from concourse.alu_op_type import AluOpType as op

    f32 = mybir.dt.float32
    f16 = mybir.dt.float16
    i32 = mybir.dt.int32
    Act = mybir.ActivationFunctionType

    cops = _register_custom_ops()
    nc = bacc.Bacc("TRN2", target_bir_lowering=False, debug=False)

    pix_d = nc.dram_tensor("pix", [128, 4, 330], f32, kind="ExternalInput").ap()
    consts_d = nc.dram_tensor("consts", [128, NCONST], f32, kind="ExternalInput").ap()
    hm_d = nc.dram_tensor("hm", [128, 10], f16, kind="ExternalInput").ap()
    confpad_d = nc.dram_tensor("confpad", [NN, 66, 322], f16, kind="ExternalInput").ap()
    # mask pre-packed on host to [gc, wc, (nn,r), k, q2, w] fp16 (k-major planes)
    mask_d = nc.dram_tensor("maskpk", [4, 2, 128, KQ], f16, kind="ExternalInput").ap()
    out_d = nc.dram_tensor("out", [RPC * UP, W * UP], f32, kind="ExternalOutput").ap()

    def dram_ap(base, off, dims):
        return bass.AP(tensor=base.tensor, offset=base.offset + off, ap=[list(d) for d in dims])

    def sap(a, off, dims):
        """Manual SBUF AP: keep the tile's partition dim, custom free dims."""
        return bass.AP(
            tensor=a.tensor, offset=a.offset + off,
            ap=[list(a.ap[0])] + [list(d) for d in dims],
        )

    with tile.TileContext(nc) as tc:
        with tc.tile_pool(name="persist", bufs=1) as pp:
            consts = pp.tile([128, NCONST], f32, name="consts")
            nc.sync.dma_start(out=consts[:], in_=consts_d)

            def CC(i, p0=0, p1=128):
                return consts[p0:p1, i : i + 1]

            pix = pp.tile([128, 4, 330], f32, name="pix")
            nc.sync.dma_start(out=pix[:], in_=pix_d)
            hm = pp.tile([128, 10], f16, name="hm")
            nc.sync.dma_start(out=hm[:], in_=hm_d)

            t3i = pp.tile([128, 3, 322], f16, name="t3i")
            t3c = pp.tile([128, 3, 322], f16, name="t3c")
            ufi = pp.tile([128, 9, 320], f16, name="ufi")
            ufc = pp.tile([128, 9, 320], f16, name="ufc")
            inv16 = pp.tile([128, 330], f16, name="inv16")

            # conf unfold rows straight from the host-padded input
            for nn in range(NN):
                src = dram_ap(
                    confpad_d, nn * 66 * 322,
                    [[322, 64], [322, 3], [1, 322]],
                )
                nc.sync.dma_start(out=t3c[nn * 64 : nn * 64 + 64], in_=src)
            # build conf tap planes early (independent of geometry)
            for dy in range(3):
                nc.vector.tensor_scalar(
                    out=sap(ufc[:], 3 * dy * 320, [[320, 3], [1, 320]]),
                    in0=sap(t3c[:], dy * 322, [[1, 3], [1, 320]]),
                    scalar1=0.0, scalar2=None, op0=op.bypass,
                )
            # zero t3i (cols 0/321 stay 0; rows overwritten below)
            nc.vector.memset(t3i[:], 0.0)

            # ---------------- geometry (fp32, 3-engine spread) ----------------
            u = pix[:, 0, :]
            v = pix[:, 1, :]
            d = pix[:, 2, :]
            fl = pix[:, 3, :]

            with tc.tile_pool(name="geom", bufs=1) as gp:
                _tagn = [0]

                def T(dt=f32):
                    _tagn[0] += 1
                    return gp.tile([128, 330], dt, name=f"g{_tagn[0]}", tag=f"g{_tagn[0]}")

                def TT(eng, o, a, b, alu):
                    eng.tensor_tensor(out=o, in0=a, in1=b, op=alu)

                def TS(eng, o, a, s1, o0, s2=None, o1=None):
                    if o1 is None:
                        eng.tensor_scalar(out=o, in0=a, scalar1=s1, scalar2=None, op0=o0)
                    else:
                        eng.tensor_scalar(out=o, in0=a, scalar1=s1, scalar2=s2, op0=o0, op1=o1)

                def STT(eng, o, a, s, b, o0, o1):
                    eng.scalar_tensor_tensor(out=o, in0=a, scalar=s, in1=b, op0=o0, op1=o1)

                def AF(o, a, scale, bias, func=Act.Identity):
                    nc.scalar.activation(out=o, in_=a, func=func, scale=scale, bias=bias)

                def AB(o, a):
                    nc.scalar.activation(out=o, in_=a, func=Act.Abs)

                def recip_acc(o, x):
                    t = T()
                    nc.vector.reciprocal_approx_accurate(out=o, in_=x, scratch=t[:])

                V, G = nc.vector, nc.gpsimd

                # linear forms of (u, v): a_j = M @ [u, v, 1], r_j = R @ [u, v, 1]
                a0, a1, a2 = T(), T(), T()
                rx, ry, rz = T(), T(), T()
                t1_, t2_, t3_, t4_, t5_, t6_ = T(), T(), T(), T(), T(), T()
                AF(t1_[:], u, CC(C_M20), CC(C_M22))
                STT(V, a2[:], v, CC(C_M21), t1_[:], op.mult, op.add)
                AF(t2_[:], u, CC(C_R20), CC(C_R22))
                STT(V, rz[:], v, CC(C_R21), t2_[:], op.mult, op.add)
                AF(t3_[:], u, CC(C_M00), CC(C_M02))
                STT(V, a0[:], v, CC(C_M01), t3_[:], op.mult, op.add)
                AF(t4_[:], u, CC(C_M10), CC(C_M12))
                STT(V, a1[:], v, CC(C_M11), t4_[:], op.mult, op.add)
                AF(t5_[:], u, CC(C_R00), CC(C_R02))
                STT(V, rx[:], v, CC(C_R01), t5_[:], op.mult, op.add)
                AF(t6_[:], u, CC(C_R10), CC(C_R12))
                STT(V, ry[:], v, CC(C_R11), t6_[:], op.mult, op.add)

                d10 = T()
                AF(d10[:], d, 1.0, CC(C_TEN))

                # z components and their reciprocals
                ps2, pe2, rs2, re2 = T(), T(), T(), T()
                m1, m2, ab1, ab2 = T(), T(), T(), T()
                TT(V, m1[:], a2[:], d, op.mult)
                AF(ps2[:], m1[:], 1.0, CC(C_T2))
                TT(V, m2[:], a2[:], d10[:], op.mult)
                TT(V, pe2[:], m2[:], ps2[:], op.add)
                AB(ab1[:], ps2[:])
                TS(V, ab1[:], ab1[:], EPS, op.add)
                recip_acc(rs2[:], ab1[:])
                AB(ab2[:], pe2[:])
                TS(V, ab2[:], ab2[:], EPS, op.add)
                recip_acc(re2[:], ab2[:])

                # x/y projections (start and end)
                pxs, pxe, pys, pye = T(), T(), T(), T()
                mx1, mx2, psx, pex = T(), T(), T(), T()
                TT(V, mx1[:], a0[:], d, op.mult)
                AF(psx[:], mx1[:], 1.0, CC(C_T0))
                TT(V, mx2[:], a0[:], d10[:], op.mult)
                TT(V, pex[:], mx2[:], psx[:], op.add)
                TT(V, pxs[:], psx[:], rs2[:], op.mult)
                TT(V, pxe[:], pex[:], re2[:], op.mult)
                my1, my2, psy, pey = T(), T(), T(), T()
                TT(V, my1[:], a1[:], d, op.mult)
                AF(psy[:], my1[:], 1.0, CC(C_T1))
                TT(V, my2[:], a1[:], d10[:], op.mult)
                TT(V, pey[:], my2[:], psy[:], op.add)
                TT(V, pys[:], psy[:], rs2[:], op.mult)
                TT(V, pye[:], pey[:], re2[:], op.mult)

                fdx, fdy = T(), T()
                TT(V, fdx[:], pxe[:], pxs[:], op.subtract)
                TT(V, fdy[:], pye[:], pys[:], op.subtract)

                # rsqrt(fdx^2 + fdy^2) via magic seed + 2 fused Newton steps
                q = T()
                nc.vector._custom_dve(cops["SUMSQ_ANT"], out=q[:], in0=fdx[:], in1=fdy[:])
                y = T()
                yi = y[:].bitcast(i32)
                TS(V, yi, q[:].bitcast(i32), 1, op.arith_shift_right)
                TS(V, yi, yi, -1, op.bitwise_xor)
                TS(V, yi, yi, 0x5F3759DF + 1, op.add)
                y2 = T()
                nc.vector._custom_dve(
                    cops["RSQRT_NR_ANT"], out=y2[:], in0=y[:], in1=q[:], s0=-0.5, s1=1.5
                )
                nc.vector._custom_dve(
                    cops["RSQRT_NR_ANT"], out=y[:], in0=y2[:], in1=q[:], s0=-0.5, s1=1.5
                )

                fls, mx, my, mxt, myt = T(), T(), T(), T(), T()
                TT(V, fls[:], fl, y[:], op.mult)
                TT(V, mxt[:], fdx[:], fls[:], op.mult)
                TT(V, mx[:], mxt[:], pxs[:], op.add)
                TT(V, myt[:], fdy[:], fls[:], op.mult)
                TT(V, my[:], myt[:], pys[:], op.add)

                ax, ay, fm = T(), T(), T()
                fmi = fm[:].bitcast(i32)
                AB(ax[:], fdx[:])
                AB(ay[:], fdy[:])
                TT(V, fmi, ay[:], ax[:], op.is_gt)

                nx, ny, tnx, tny = T(), T(), T(), T()
                AF(tnx[:], mx[:], CC(C_A0), CC(C_A2))
                STT(V, nx[:], my[:], CC(C_A1), tnx[:], op.mult, op.add)
                AF(tny[:], mx[:], CC(C_B0), CC(C_B2))
                STT(V, ny[:], my[:], CC(C_B1), tny[:], op.mult, op.add)

                def inv_axis(eng, o, nj, rj, c_t):
                    z1, nm, dn, rden = T(), T(), T(), T()
                    TT(eng, z1[:], rz[:], nj[:], op.mult)
                    TT(eng, z1[:], rj[:], z1[:], op.subtract)
                    AB(nm[:], z1[:])
                    # |nj*tz + (-t_j)| + eps, affine folded into the Abs table
                    nc.scalar.activation(
                        out=dn[:], in_=nj[:], func=Act.Abs,
                        scale=CC(C_TZ), bias=CC(c_t),
                    )
                    TS(eng, dn[:], dn[:], EPS, op.add)
                    recip_acc(rden[:], dn[:])
                    TT(eng, o, nm[:], rden[:], op.mult)

                invx, invy = T(), T()
                inv_axis(V, invx[:], nx, rx, C_TX)
                inv_axis(G, invy[:], ny, ry, C_TY)

                seld, selA = T(), T()
                nc.vector.select(out=seld[:], mask=fmi, on_true=invy[:], on_false=invx[:])
                AF(selA[:], seld[:], CC(C_CA), CC(C_CB))
                TS(V, inv16[:], selA[:], 0.0, op.max, 1.0, op.min)

            # zero the halo pixels that fall outside the image (edge chunks)
            nc.vector.tensor_tensor(
                out=inv16[:, 320:330], in0=inv16[:, 320:330], in1=hm[:], op=op.mult
            )

            # ---- on-chip 3-row halo build (SBUF->SBUF DMAs, no DRAM trip) ----
            # t3i[p, 1, 1:321] = inv row p
            nc.vector.tensor_scalar(
                out=sap(t3i[:], 322 + 1, [[1, 320]]),
                in0=sap(inv16[:], 0, [[1, 320]]),
                scalar1=0.0, scalar2=None, op0=op.bypass,
            )
            for nn in range(NN):
                b0 = nn * 64
                # row above: shift partitions down by one
                nc.sync.dma_start(
                    out=sap(t3i[b0 + 1 : b0 + 64], 1, [[1, 320]]),
                    in_=sap(inv16[b0 : b0 + 63], 0, [[1, 320]]),
                )
                # top halo row (packed 5 cols/partition across the block)
                nc.sync.dma_start(
                    out=sap(t3i[b0 : b0 + 1], 1, [[1, 320]]),
                    in_=inv16[b0 : b0 + 64, 320:325],
                )
                # row below: shift partitions up by one
                nc.sync.dma_start(
                    out=sap(t3i[b0 : b0 + 63], 2 * 322 + 1, [[1, 320]]),
                    in_=sap(inv16[b0 + 1 : b0 + 64], 0, [[1, 320]]),
                )
                # bottom halo row
                nc.sync.dma_start(
                    out=sap(t3i[b0 + 63 : b0 + 64], 2 * 322 + 1, [[1, 320]]),
                    in_=inv16[b0 : b0 + 64, 325:330],
                )
            # inv tap planes
            for dy in range(3):
                nc.vector.tensor_scalar(
                    out=sap(ufi[:], 3 * dy * 320, [[320, 3], [1, 320]]),
                    in0=sap(t3i[:], dy * 322, [[1, 3], [1, 320]]),
                    scalar1=0.0, scalar2=None, op0=op.bypass,
                )

            # ---------------- chunk loop ----------------
            with tc.tile_pool(name="chunk", bufs=2) as cp, tc.tile_pool(
                name="chunk1", bufs=1
            ) as cp1:
                tails = []

                def emit_tail(t):
                    a32, gc, w0 = t
                    # write (q2, w) input as (w, q2)-interleaved so the output
                    # DMA is one contiguous 640-run per partition
                    outt = cp.tile([64, UP * WC], f32, name="outt", tag="outt")
                    nc.vector.reciprocal_approx_fast(
                        out=sap(outt[:], 0, [[1, UP], [UP, WC]]),
                        in_=sap(a32[:], 0, [[WC, UP], [1, WC]]),
                    )
                    dst = dram_ap(
                        out_d, gc * (W * UP) + UP * w0,
                        [[UP * W * UP, 64], [1, UP * WC]],
                    )
                    nc.sync.dma_start(out=dst, in_=outt[:])

                for ci, (wc, gc) in enumerate(
                    [(w_, g_) for w_ in range(2) for g_ in range(4)]
                ):
                    w0 = wc * WC
                    M = cp.tile([128, 9, 4 * WC], f16, name="m", tag="m")
                    nc.sync.dma_start(out=M[:], in_=mask_d[gc, wc])
                    EM = cp.tile([128, 27, 4 * WC], f16, name="em", tag="em")
                    # e = exp(mask) -> planes 0-8
                    nc.scalar.activation(out=EM[:, 0:9, :], in_=M[:], func=Act.Exp)
                    # e * uf tap products -> planes 9-17 (inv), 18-26 (conf)
                    for tg, uf in ((1, ufi), (2, ufc)):
                        nc.vector.tensor_tensor(
                            out=sap(EM[:], tg * 9 * 640, [[640, 9], [WC, 4], [1, WC]]),
                            in0=sap(EM[:], 0, [[640, 9], [WC, 4], [1, WC]]),
                            in1=sap(uf[:], w0, [[320, 9], [0, 4], [1, WC]]),
                            op=op.mult,
                        )
                    # joint 9-tap sum tree over tags (e, e*ufi, e*ufc)
                    t1 = cp1.tile([128, 12, 640], f16, name="t1", tag="t1")
                    nc.vector.tensor_tensor(
                        out=sap(t1[:], 0, [[2560, 3], [640, 4], [1, 640]]),
                        in0=sap(EM[:], 0, [[5760, 3], [1280, 4], [1, 640]]),
                        in1=sap(EM[:], 640, [[5760, 3], [1280, 4], [1, 640]]),
                        op=op.add,
                    )
                    t2 = cp1.tile([128, 6, 640], f16, name="t2", tag="t2")
                    nc.vector.tensor_tensor(
                        out=sap(t2[:], 0, [[1280, 3], [640, 2], [1, 640]]),
                        in0=sap(t1[:], 0, [[2560, 3], [1280, 2], [1, 640]]),
                        in1=sap(t1[:], 640, [[2560, 3], [1280, 2], [1, 640]]),
                        op=op.add,
                    )
                    t3 = cp1.tile([128, 3, 640], f16, name="t3", tag="t3")
                    nc.vector.tensor_tensor(
                        out=sap(t3[:], 0, [[640, 3], [1, 640]]),
                        in0=sap(t2[:], 0, [[1280, 3], [1, 640]]),
                        in1=sap(t2[:], 640, [[1280, 3], [1, 640]]),
                        op=op.add,
                    )
                    # level 4: add the 9th tap
                    numic = cp.tile([128, 2, 640], f16, name="numic", tag="numic")
                    nc.vector.tensor_tensor(
                        out=sap(numic[:], 0, [[640, 2], [1, 640]]),
                        in0=sap(t3[:], 640, [[640, 2], [1, 640]]),
                        in1=sap(EM[:], 17 * 640, [[5760, 2], [1, 640]]),
                        op=op.add,
                    )
                    s32 = cp.tile([128, 640], f32, name="s32", tag="s32")
                    nc.vector.tensor_tensor(
                        out=s32[:], in0=t3[:, 0, :], in1=EM[:, 8, :], op=op.add
                    )
                    rs32 = cp.tile([128, 640], f32, name="rs32", tag="rs32")
                    nc.vector.reciprocal_approx_fast(out=rs32[:], in_=s32[:])
                    iu = cp.tile([128, 640], f16, name="iu", tag="iu")
                    cu = cp.tile([128, 640], f16, name="cu", tag="cu")
                    nc.vector.tensor_tensor(
                        out=iu[:], in0=numic[:, 0, :], in1=rs32[:], op=op.mult
                    )
                    nc.vector.tensor_tensor(
                        out=cu[:], in0=numic[:, 1, :], in1=rs32[:], op=op.mult
                    )
                    # move nn1 halves down to partitions 0-63 for the 2-view fusion
                    iu2 = cp.tile([64, 640], f16, name="iu2", tag="iu2")
                    cu2 = cp.tile([64, 640], f16, name="cu2", tag="cu2")
                    nc.sync.dma_start(out=iu2[:], in_=iu[64:128])
                    nc.sync.dma_start(out=cu2[:], in_=cu[64:128])
                    lo = slice(0, 64)

                    def F(tag):
                        return cp.tile([64, 640], f16, name="f" + tag, tag="f" + tag)

                    # w1 = sigmoid(cu1-cu0) via tanh; fused*2 = (iu0+iu1) + t*(iu1-iu0)
                    dl, tt, di, sm = F("dl"), F("tt"), F("di"), F("sm")
                    nc.vector.tensor_tensor(out=dl[:], in0=cu2[:], in1=cu[lo], op=op.subtract)
                    nc.scalar.activation(out=tt[:], in_=dl[:], func=Act.Tanh, scale=0.5)
                    nc.vector.tensor_tensor(out=di[:], in0=iu2[:], in1=iu[lo], op=op.subtract)
                    nc.vector.tensor_tensor(out=sm[:], in0=iu2[:], in1=iu[lo], op=op.add)
                    nc.vector.tensor_tensor(out=di[:], in0=tt[:], in1=di[:], op=op.mult)
                    nc.vector.tensor_tensor(out=sm[:], in0=di[:], in1=sm[:], op=op.add)
                    # a = g*0.5*(dmin-dmax) + dmax  (then out = 1/a in the tail)
                    a32 = cp.tile([64, 640], f32, name="a32", tag="a32")
                    nc.scalar.activation(
                        out=a32[:], in_=sm[:], func=Act.Identity,
                        scale=CC(C_DS2, 0, 64), bias=CC(C_DB, 0, 64),
                    )
                    tails.append((a32, gc, w0))
                    if ci >= 1:
                        emit_tail(tails[ci - 1])
                emit_tail(tails[-1])

    nc.finalize()
    return nc


def _host_prep(inputs):
    K_ref = np.asarray(inputs["K_ref"], np.float32)
    K_nei = np.asarray(inputs["K_nei"], np.float32)
    R_nei = np.asarray(inputs["R_nei"], np.float32)
    T_nei = np.asarray(inputs["T_nei"], np.float32)
    depth0 = np.asarray(inputs["depth0"], np.float32)
    flow = np.asarray(inputs["flow"], np.float32)
    mask = np.asarray(inputs["mask"], np.float32)
    conf = np.asarray(inputs["conf"], np.float32)
    dmin = float(np.asarray(inputs["depth_min"]).reshape(-1)[0])
    dmax = float(np.asarray(inputs["depth_max"]).reshape(-1)[0])

    # pixel rays per batch (u, v with unit z)
    uv = []
    for b in range(B):
        Ki = np.linalg.inv(K_ref[b, 0, 0].astype(np.float64))
        gx, gy = np.meshgrid(np.arange(W, dtype=np.float64), np.arange(H, dtype=np.float64))
        x = Ki[0, 0] * gx + Ki[0, 1] * gy + Ki[0, 2]
        y = Ki[1, 0] * gx + Ki[1, 1] * gy + Ki[1, 2]
        z = Ki[2, 0] * gx + Ki[2, 1] * gy + Ki[2, 2]
        uv.append((np.float32(x / z), np.float32(y / z)))

    cA = 1.0 / (dmin - dmax)
    cB = -dmax / (dmin - dmax)

    in_maps = []
    for c in range(NCORES):
        b, rc = c // 4, c % 4
        r0 = rc * RPC
        rtop = max(r0 - 1, 0)
        rbot = min(r0 + RPC, H - 1)

        consts = np.zeros((128, NCONST), np.float32)
        for nn in range(NN):
            Kn = K_nei[nn, b, 0, 0].astype(np.float64)
            Rn = R_nei[nn, b, 0, 0].astype(np.float64)
            Tn = T_nei[nn, b, 0, 0].astype(np.float64).reshape(3)
            M = Kn @ Rn
            t = (Kn @ Tn.reshape(3, 1)).reshape(3)
            iK = np.linalg.inv(Kn)
            assert abs(iK[2, 0]) < 1e-12 and abs(iK[2, 1]) < 1e-12 and abs(iK[2, 2] - 1) < 1e-9
            row = np.zeros(NCONST, np.float32)
            row[C_M00:C_M22 + 1] = M.reshape(-1)
            row[C_T0:C_T2 + 1] = t
            row[C_R00:C_R22 + 1] = Rn.reshape(-1)
            row[C_A0:C_A2 + 1] = iK[0] / (1.0 + EPS)
            row[C_B0:C_B2 + 1] = iK[1] / (1.0 + EPS)
            # C_TX/C_TY feed |tz*n + c| as ACT affine bias -> store negated
            row[C_TX], row[C_TY], row[C_TZ] = -Tn[0], -Tn[1], Tn[2]
            row[C_CA], row[C_CB] = cA, cB
            row[C_TEN] = 10.0
            row[C_DS2], row[C_DB] = 0.5 * (dmin - dmax), dmax
            consts[nn * 64 : nn * 64 + 64] = row

        u_full, v_full = uv[b]
        d_full = depth0[b, 0]

        pix = np.zeros((128, 4, 330), np.float32)
        for nn in range(NN):
            sl = slice(nn * 64, nn * 64 + 64)
            f_full = flow[nn, b, 0]
            for ch, arr in enumerate((u_full, v_full, d_full, f_full)):
                pix[sl, ch, 0:320] = arr[r0 : r0 + RPC]
                pix[sl, ch, 320:325] = arr[rtop].reshape(64, 5)
                pix[sl, ch, 325:330] = arr[rbot].reshape(64, 5)

        hm = np.ones((128, 10), np.float16)
        if r0 == 0:
            hm[:, 0:5] = 0.0
        if r0 + RPC == H:
            hm[:, 5:10] = 0.0

        confpad = np.zeros((NN, 66, 322), np.float16)
        confpad[:, 1:65, 1:321] = conf[:, b, 0, r0 : r0 + RPC, :]
        if r0 > 0:
            confpad[:, 0, 1:321] = conf[:, b, 0, r0 - 1, :]
        if r0 + RPC < H:
            confpad[:, 65, 1:321] = conf[:, b, 0, r0 + RPC, :]

        # [nn, k, q1, q2, r, wc, w] -> [q1, wc, (nn, r), k, q2, w]
        ms = mask[:, b, :, r0 : r0 + RPC, :].reshape(NN, 9, 4, 4, RPC, 2, WC)
        mask_pk = np.ascontiguousarray(ms.transpose(2, 5, 0, 4, 1, 3, 6)).reshape(
            4, 2, 128, KQ
        ).astype(np.float16)

        in_maps.append(
            {
                "pix": pix,
                "consts": consts,
                "hm": hm,
                "confpad": confpad,
                "maskpk": mask_pk,
            }
        )
    return in_maps


def kernel(**inputs):
    if "nc" not in _cache:
        _cache["nc"] = _build_program()
    nc = _cache["nc"]
    in_maps = _host_prep(inputs)

    from concourse import bass_utils

    res = bass_utils.run_bass_kernel_spmd(nc, in_maps, core_ids=list(range(NCORES)))
    out = np.empty((B, 1, H * UP, W * UP), np.float32)
    for c in range(NCORES):
        b, rc = c // 4, c % 4
        out[b, 0, rc * RPC * UP : (rc + 1) * RPC * UP, :] = res.results[c]["out"]
    return out


# revision 7
# speedup vs baseline: 1.4339x; 1.0423x over previous
"""DispMVS depth-fusion kernel for 8 Trainium2 NeuronCores.

Sharding: core c handles batch b = c // 4 and coarse rows r0 = (c % 4) * 64
(64 of 256 rows), with BOTH neighbor streams (NN=2) resident on the core
(partitions = nn*64 + row).  The cross-neighbor confidence-fusion softmax is
then core-local; cores never communicate.

Pipeline per core (one Bass/Tile program, identical for all 8 cores):
  1. prerun: mask DMAs + exp(mask) for the first two chunks are emitted ahead
     of geometry so the ACT engine works while the DVE runs the geometry chain
     (separate tile pools avoid the SBUF-reuse barrier).
  2. geometry: elementwise epipolar math -> clipped inverse depth (fp32 math,
     fp16 result) on DVE+ACT; denominators here are provably bounded away
     from 0, so the reference's abs/eps guards are algebraically dropped.
  3. on-chip halo build: SBUF->SBUF DMAs create 3 vertically shifted rows of
     inv-depth (t3i); conf rows come host-padded (t3c).  Shifted copies build
     the per-tap operand planes ufi/ufc [128, 9, 320] fp16.
  4. chunk loop (4 row-subpixel groups x 2 w-halves), all mask-sized work in
     fp16 with k-major planes [128, 9k, 4q, 160w]:
       ACT  exp(mask), tanh-based 2-view softmax weight, output affine
       DVE  e*ufi, e*ufc (fp16 TT at 2x), joint 3-tag 9-sum tree,
            reciprocals, weighted avgs, fusion TTs
     Tails (final recip + output DMA) are deferred one chunk so cross-engine
     latency never stalls the DVE queue.
"""

import numpy as np

NN, B, H, W = 2, 2, 256, 320
UP = 4
EPS = 1e-6
RPC = 64          # coarse rows per core
NCORES = 8
WC = 160          # w-half width
KQ = 9 * 4 * WC   # elements per chunk per partition (5760)
PL = 4 * WC       # elements per k-plane (640)

# consts columns
(
    C_M00, C_M01, C_M02, C_M10, C_M11, C_M12, C_M20, C_M21, C_M22,
    C_T0, C_T1, C_T2,
    C_R00, C_R01, C_R02, C_R10, C_R11, C_R12, C_R20, C_R21, C_R22,
    C_A0, C_A1, C_A2, C_B0, C_B1, C_B2,
    C_TX, C_TY, C_TZ,
    C_CA, C_CB, C_DS2, C_DB, C_TEN,
) = range(35)
NCONST = 36

_cache = {}


def _register_custom_ops():
    """Register this kernel's custom DVE ops (idempotent). Returns a dict.

    SUMSQ_ANT: out = in0^2 + in1^2 (one pass instead of 3).
    RSQRT_NR_ANT: one Newton step for 1/sqrt: out = in0*(1.5 - 0.5*in1*in0^2)
      (one pass instead of 4).
    MUL_CUMSUM_ANT kept for sub-opcode stability with earlier builds.
    """
    from concourse import dve_ops
    from concourse.dve_spec import AluOp, C0, C1, Spec, Src0, Src1, _has_src1, lower, scan
    from concourse.dve_uop import DveOpSpec

    have = {o.name: o for o in dve_ops.OPS}
    if "MUL_CUMSUM_ANT" in have:
        return have

    def cum_ref(in0, in1, s0, s1, imm2):
        a = in0.astype(np.float32).reshape(in0.shape[0], -1) * in1.astype(
            np.float32
        ).reshape(in1.shape[0], -1)
        return np.cumsum(a, axis=1, dtype=np.float32).reshape(in0.shape)

    specs = [
        ("MUL_CUMSUM_ANT", Spec(body=scan(AluOp.ADD, Src0 * Src1), reference=cum_ref)),
        (
            "SUMSQ_ANT",
            Spec(
                body=Src0 * Src0 + Src1 * Src1,
                reference=lambda in0, in1, s0, s1, imm2: (
                    in0.astype(np.float32) ** 2 + in1.astype(np.float32) ** 2
                ),
            ),
        ),
        (
            "RSQRT_NR_ANT",
            Spec(
                body=(Src0 * Src0 * Src1 * C0 + C1) * Src0,
                reference=lambda in0, in1, s0, s1, imm2: (
                    (in0.astype(np.float32) ** 2 * in1 * s0 + s1) * in0
                ),
            ),
        ),
    ]
    out = {}
    for name, spec in specs:
        op_ = dve_ops.DveOp(name, spec, subdim=False, uops_sha={})
        dve_ops.OPS.append(op_)
        dve_ops.CUSTOM_DVE_SPECS[name] = spec
        dve_ops._SUB_OPCODE_FOR_NAME[name] = (
            dve_ops._CUSTOM_DVE_ROW_BASE + len(dve_ops.OPS) - 1
        )
        for ver in ("v3", "v4"):
            tmp = DveOpSpec(
                name=name,
                opcode=dve_ops.get_dve_sub_opcode(name),
                uops=lower(spec, ver=ver),
                rd1_en=_has_src1(spec),
            )
            op_.uops_sha[ver] = tmp.sha(ver)
        out[name] = op_
    assert max(dve_ops._SUB_OPCODE_FOR_NAME.values()) < 0x20
    return out


def _build_program():
    import concourse.bass as bass
    import concourse.bacc as bacc
    import concourse.tile as tile
    from concourse import mybir
    from concourse.alu_op_type import AluOpType as op
    from concourse.dve_ops import RECIP_APPROX_FAST_CONSTS, RECIPROCAL_APPROX_FAST

    f32 = mybir.dt.float32
    f16 = mybir.dt.float16
    i32 = mybir.dt.int32
    Act = mybir.ActivationFunctionType

    cops = _register_custom_ops()
    nc = bacc.Bacc("TRN2", target_bir_lowering=False, debug=False)

    pix_d = nc.dram_tensor("pix", [128, 4, 330], f32, kind="ExternalInput").ap()
    consts_d = nc.dram_tensor("consts", [128, NCONST], f32, kind="ExternalInput").ap()
    hm_d = nc.dram_tensor("hm", [128, 10], f16, kind="ExternalInput").ap()
    confpad_d = nc.dram_tensor("confpad", [NN, 66, 322], f16, kind="ExternalInput").ap()
    # mask pre-packed on host to [gc, wc, (nn,r), k, q2, w] fp16 (k-major planes)
    mask_d = nc.dram_tensor("maskpk", [4, 2, 128, KQ], f16, kind="ExternalInput").ap()
    out_d = nc.dram_tensor("out", [RPC * UP, W * UP], f32, kind="ExternalOutput").ap()

    def dram_ap(base, off, dims):
        return bass.AP(tensor=base.tensor, offset=base.offset + off, ap=[list(d) for d in dims])

    def sap(a, off, dims):
        """Manual SBUF AP: keep the tile's partition dim, custom free dims."""
        return bass.AP(
            tensor=a.tensor, offset=a.offset + off,
            ap=[list(a.ap[0])] + [list(d) for d in dims],
        )

    CHUNKS = [(w_, g_) for w_ in range(2) for g_ in range(4)]

    with tile.TileContext(nc) as tc:
        with tc.tile_pool(name="persist", bufs=1) as pp, tc.tile_pool(
            name="io", bufs=2
        ) as io:
            consts = pp.tile([128, NCONST], f32, name="consts")
            nc.sync.dma_start(out=consts[:], in_=consts_d)

            def CC(i, p0=0, p1=128):
                return consts[p0:p1, i : i + 1]

            pix = pp.tile([128, 4, 330], f32, name="pix")
            nc.sync.dma_start(out=pix[:], in_=pix_d)
            hm = pp.tile([128, 10], f16, name="hm")
            nc.sync.dma_start(out=hm[:], in_=hm_d)

            t3i = pp.tile([128, 3, 322], f16, name="t3i")
            t3c = pp.tile([128, 3, 322], f16, name="t3c")
            ufi = pp.tile([128, 9, 320], f16, name="ufi")
            ufc = pp.tile([128, 9, 320], f16, name="ufc")
            inv16 = pp.tile([128, 330], f16, name="inv16")

            # conf unfold rows straight from the host-padded input
            for nn in range(NN):
                src = dram_ap(
                    confpad_d, nn * 66 * 322,
                    [[322, 64], [322, 3], [1, 322]],
                )
                nc.sync.dma_start(out=t3c[nn * 64 : nn * 64 + 64], in_=src)
            # build conf tap planes early (independent of geometry)
            for dy in range(3):
                nc.vector.tensor_scalar(
                    out=sap(ufc[:], 3 * dy * 320, [[320, 3], [1, 320]]),
                    in0=sap(t3c[:], dy * 322, [[1, 3], [1, 320]]),
                    scalar1=0.0, scalar2=None, op0=op.bypass,
                )
            # zero t3i (cols 0/321 stay 0; rows overwritten below)
            nc.vector.memset(t3i[:], 0.0)

            # ---- prerun: mask DMA + exp for the first two chunks ----
            def emit_head(ci):
                gc, wc = CHUNKS[ci][1], CHUNKS[ci][0]
                M = io.tile([128, 9, PL], f16, name="m", tag="m")
                nc.sync.dma_start(out=M[:], in_=mask_d[gc, wc])
                EM = io.tile([128, 27, PL], f16, name="em", tag="em")
                nc.scalar.activation(out=EM[:, 0:9, :], in_=M[:], func=Act.Exp)
                return EM

            pre = [emit_head(0), emit_head(1)]

            # ---------------- geometry (fp32, DVE+ACT) ----------------
            u = pix[:, 0, :]
            v = pix[:, 1, :]
            d = pix[:, 2, :]
            fl = pix[:, 3, :]

            with tc.tile_pool(name="geom", bufs=1) as gp:
                _tagn = [0]

                def T():
                    _tagn[0] += 1
                    return gp.tile([128, 330], f32, name=f"g{_tagn[0]}", tag=f"g{_tagn[0]}")

                V = nc.vector

                def TT(o, a, b, alu):
                    V.tensor_tensor(out=o, in0=a, in1=b, op=alu)

                def TS(o, a, s1, o0, s2=None, o1=None):
                    if o1 is None:
                        V.tensor_scalar(out=o, in0=a, scalar1=s1, scalar2=None, op0=o0)
                    else:
                        V.tensor_scalar(out=o, in0=a, scalar1=s1, scalar2=s2, op0=o0, op1=o1)

                def STT(o, a, s, b, o0, o1):
                    V.scalar_tensor_tensor(out=o, in0=a, scalar=s, in1=b, op0=o0, op1=o1)

                def AF(o, a, scale, bias, func=Act.Identity):
                    nc.scalar.activation(out=o, in_=a, func=func, scale=scale, bias=bias)

                def AB(o, a):
                    nc.scalar.activation(out=o, in_=a, func=Act.Abs)

                def RF(o, x):
                    V.reciprocal_approx_fast(out=o, in_=x)

                # linear forms of (u, v): a_j = (K@R) @ [u,v,1], r_j = R @ [u,v,1]
                a0, a1, a2 = T(), T(), T()
                rx, ry, rz = T(), T(), T()
                t1_, t2_, t3_, t4_, t5_, t6_, t7_ = T(), T(), T(), T(), T(), T(), T()
                AF(t1_[:], u, CC(C_M20), CC(C_M22))
                STT(a2[:], v, CC(C_M21), t1_[:], op.mult, op.add)
                AF(t2_[:], u, CC(C_M00), CC(C_M02))
                STT(a0[:], v, CC(C_M01), t2_[:], op.mult, op.add)
                AF(t3_[:], u, CC(C_M10), CC(C_M12))
                STT(a1[:], v, CC(C_M11), t3_[:], op.mult, op.add)
                AF(t4_[:], u, CC(C_R20), CC(C_R22))
                STT(rz[:], v, CC(C_R21), t4_[:], op.mult, op.add)
                AF(t5_[:], u, CC(C_R00), CC(C_R02))
                STT(rx[:], v, CC(C_R01), t5_[:], op.mult, op.add)
                AF(t6_[:], u, CC(C_R10), CC(C_R12))
                STT(ry[:], v, CC(C_R11), t6_[:], op.mult, op.add)
                d10 = T()
                AF(d10[:], d, 1.0, CC(C_TEN))

                # z components (always positive here) and reciprocals
                ps2, pe2, rs2, re2, m1, m2 = T(), T(), T(), T(), T(), T()
                TT(m1[:], a2[:], d, op.mult)
                AF(ps2[:], m1[:], 1.0, CC(C_T2))
                RF(rs2[:], ps2[:])
                TT(m2[:], a2[:], d10[:], op.mult)
                TT(pe2[:], m2[:], ps2[:], op.add)
                RF(re2[:], pe2[:])

                # x/y projections (start and end)
                pxs, pxe, pys, pye = T(), T(), T(), T()
                psx, psy = T(), T()
                TT(t1_[:], a0[:], d, op.mult)
                AF(psx[:], t1_[:], 1.0, CC(C_T0))
                TT(pxs[:], psx[:], rs2[:], op.mult)
                TT(t2_[:], a0[:], d10[:], op.mult)
                TT(t2_[:], t2_[:], psx[:], op.add)
                TT(pxe[:], t2_[:], re2[:], op.mult)
                TT(t3_[:], a1[:], d, op.mult)
                AF(psy[:], t3_[:], 1.0, CC(C_T1))
                TT(pys[:], psy[:], rs2[:], op.mult)
                TT(t4_[:], a1[:], d10[:], op.mult)
                TT(t4_[:], t4_[:], psy[:], op.add)
                TT(pye[:], t4_[:], re2[:], op.mult)

                fdx, fdy = T(), T()
                TT(fdx[:], pxe[:], pxs[:], op.subtract)
                TT(fdy[:], pye[:], pys[:], op.subtract)

                # fl/sqrt(fdx^2+fdy^2) via magic seed + 2 fused Newton steps
                q = T()
                V._custom_dve(cops["SUMSQ_ANT"], out=q[:], in0=fdx[:], in1=fdy[:])
                y, y2 = T(), T()
                yi = y[:].bitcast(i32)
                TS(yi, q[:].bitcast(i32), 1, op.arith_shift_right)
                TS(yi, yi, -1, op.bitwise_xor)
                TS(yi, yi, 0x5F3759DF + 1, op.add)
                V._custom_dve(cops["RSQRT_NR_ANT"], out=y2[:], in0=y[:], in1=q[:], s0=-0.5, s1=1.5)
                V._custom_dve(cops["RSQRT_NR_ANT"], out=y[:], in0=y2[:], in1=q[:], s0=-0.5, s1=1.5)

                fls, mx, my = T(), T(), T()
                TT(fls[:], fl, y[:], op.mult)
                TT(t5_[:], fdx[:], fls[:], op.mult)
                TT(mx[:], t5_[:], pxs[:], op.add)
                TT(t6_[:], fdy[:], fls[:], op.mult)
                TT(my[:], t6_[:], pys[:], op.add)

                ax, fm = T(), T()
                fmi = fm[:].bitcast(i32)
                AB(ax[:], fdx[:])
                AB(t7_[:], fdy[:])
                TT(fmi, t7_[:], ax[:], op.is_gt)

                nx, ny = T(), T()
                AF(t1_[:], mx[:], CC(C_A0), CC(C_A2))
                STT(nx[:], my[:], CC(C_A1), t1_[:], op.mult, op.add)
                AF(t2_[:], mx[:], CC(C_B0), CC(C_B2))
                STT(ny[:], my[:], CC(C_B1), t2_[:], op.mult, op.add)

                def inv_axis(o, nj, rj, c_t, s1, s2, s3):
                    TT(s1[:], rz[:], nj[:], op.mult)
                    TT(s1[:], rj[:], s1[:], op.subtract)
                    AB(s2[:], s1[:])          # |r_j - r_z n_j|
                    # |n_j*tz - t_j| via the Abs table's affine input
                    nc.scalar.activation(
                        out=s3[:], in_=nj[:], func=Act.Abs,
                        scale=CC(C_TZ), bias=CC(c_t),
                    )
                    RF(s1[:], s3[:])
                    TT(o, s2[:], s1[:], op.mult)

                invx, invy = T(), T()
                inv_axis(invx[:], nx, rx, C_TX, t3_, t4_, t5_)
                inv_axis(invy[:], ny, ry, C_TY, t6_, t7_, t1_)

                seld, selA = T(), T()
                V.select(out=seld[:], mask=fmi, on_true=invy[:], on_false=invx[:])
                AF(selA[:], seld[:], CC(C_CA), CC(C_CB))
                TS(inv16[:], selA[:], 0.0, op.max, 1.0, op.min)

            # zero the halo pixels that fall outside the image (edge chunks)
            nc.vector.tensor_tensor(
                out=inv16[:, 320:330], in0=inv16[:, 320:330], in1=hm[:], op=op.mult
            )

            # ---- on-chip 3-row halo build (SBUF->SBUF DMAs, no DRAM trip) ----
            nc.vector.tensor_scalar(
                out=sap(t3i[:], 322 + 1, [[1, 320]]),
                in0=sap(inv16[:], 0, [[1, 320]]),
                scalar1=0.0, scalar2=None, op0=op.bypass,
            )
            for nn in range(NN):
                b0 = nn * 64
                nc.sync.dma_start(
                    out=sap(t3i[b0 + 1 : b0 + 64], 1, [[1, 320]]),
                    in_=sap(inv16[b0 : b0 + 63], 0, [[1, 320]]),
                )
                nc.sync.dma_start(
                    out=sap(t3i[b0 : b0 + 1], 1, [[1, 320]]),
                    in_=inv16[b0 : b0 + 64, 320:325],
                )
                nc.sync.dma_start(
                    out=sap(t3i[b0 : b0 + 63], 2 * 322 + 1, [[1, 320]]),
                    in_=sap(inv16[b0 + 1 : b0 + 64], 0, [[1, 320]]),
                )
                nc.sync.dma_start(
                    out=sap(t3i[b0 + 63 : b0 + 64], 2 * 322 + 1, [[1, 320]]),
                    in_=inv16[b0 : b0 + 64, 325:330],
                )
            # inv tap planes
            for dy in range(3):
                nc.vector.tensor_scalar(
                    out=sap(ufi[:], 3 * dy * 320, [[320, 3], [1, 320]]),
                    in0=sap(t3i[:], dy * 322, [[1, 3], [1, 320]]),
                    scalar1=0.0, scalar2=None, op0=op.bypass,
                )

            # GpSimd throughput probes (results unused; engine is idle anyway)
            probe = pp.tile([128, 2880], f16, name="gprobe")
            nc.gpsimd.tensor_tensor(
                out=probe[:], in0=sap(ufc[:], 0, [[1, 2880]]),
                in1=sap(ufc[:], 0, [[1, 2880]]), op=op.mult,
            )
            nc.gpsimd.tensor_tensor(
                out=probe[:, 0:640], in0=sap(ufc[:], 0, [[1, 640]]),
                in1=sap(ufc[:], 0, [[1, 640]]), op=op.add,
            )

            # ---------------- chunk loop ----------------
            with tc.tile_pool(name="work", bufs=2) as cp, tc.tile_pool(
                name="work1", bufs=1
            ) as cp1:
                tails = []

                def emit_tail(t):
                    a32, gc, wc = t
                    outt = cp.tile([64, PL], f32, name="outt", tag="outt")
                    nc.vector.reciprocal_approx_fast(out=outt[:], in_=a32[:])
                    dst = dram_ap(
                        out_d, gc * (W * UP) + wc * PL,
                        [[UP * W * UP, 64], [1, PL]],
                    )
                    nc.sync.dma_start(out=dst, in_=outt[:])

                for ci, (wc, gc) in enumerate(CHUNKS):
                    w0 = wc * WC
                    EM = pre[ci] if ci < 2 else emit_head(ci)
                    # e * uf tap products -> planes 9-17 (inv), 18-26 (conf)
                    for tg, uf in ((1, ufi), (2, ufc)):
                        nc.vector.tensor_tensor(
                            out=sap(EM[:], tg * 9 * PL, [[PL, 9], [WC, 4], [1, WC]]),
                            in0=sap(EM[:], 0, [[PL, 9], [WC, 4], [1, WC]]),
                            in1=sap(uf[:], w0, [[320, 9], [0, 4], [1, WC]]),
                            op=op.mult,
                        )
                    # joint 9-tap sum tree over tags (e, e*ufi, e*ufc)
                    t1 = cp1.tile([128, 12, PL], f16, name="t1", tag="t1")
                    nc.vector.tensor_tensor(
                        out=sap(t1[:], 0, [[4 * PL, 3], [PL, 4], [1, PL]]),
                        in0=sap(EM[:], 0, [[9 * PL, 3], [2 * PL, 4], [1, PL]]),
                        in1=sap(EM[:], PL, [[9 * PL, 3], [2 * PL, 4], [1, PL]]),
                        op=op.add,
                    )
                    t2 = cp1.tile([128, 6, PL], f16, name="t2", tag="t2")
                    nc.vector.tensor_tensor(
                        out=sap(t2[:], 0, [[2 * PL, 3], [PL, 2], [1, PL]]),
                        in0=sap(t1[:], 0, [[4 * PL, 3], [2 * PL, 2], [1, PL]]),
                        in1=sap(t1[:], PL, [[4 * PL, 3], [2 * PL, 2], [1, PL]]),
                        op=op.add,
                    )
                    t3 = cp1.tile([128, 3, PL], f16, name="t3", tag="t3")
                    nc.vector.tensor_tensor(
                        out=sap(t3[:], 0, [[PL, 3], [1, PL]]),
                        in0=sap(t2[:], 0, [[2 * PL, 3], [1, PL]]),
                        in1=sap(t2[:], PL, [[2 * PL, 3], [1, PL]]),
                        op=op.add,
                    )
                    # level 4: add the 9th tap
                    numic = cp.tile([128, 2, PL], f16, name="numic", tag="numic")
                    nc.vector.tensor_tensor(
                        out=sap(numic[:], 0, [[PL, 2], [1, PL]]),
                        in0=sap(t3[:], PL, [[PL, 2], [1, PL]]),
                        in1=sap(EM[:], 17 * PL, [[9 * PL, 2], [1, PL]]),
                        op=op.add,
                    )
                    s32 = cp.tile([128, PL], f32, name="s32", tag="s32")
                    nc.vector.tensor_tensor(
                        out=s32[:], in0=t3[:, 0, :], in1=EM[:, 8, :], op=op.add
                    )
                    # rs = 1/s with fp16 output (custom call skips the fp32-out
                    # assert; the seed math still runs on the fp32 input)
                    rs16 = cp.tile([128, PL], f16, name="rs16", tag="rs16")
                    c = RECIP_APPROX_FAST_CONSTS
                    nc.vector._custom_dve(
                        RECIPROCAL_APPROX_FAST, out=rs16[:], in0=s32[:],
                        s0=c["s0"], s1=c["s1"], imm2=c["imm2"],
                    )
                    # weighted averages iu (inv) and cu (conf) in one op
                    iucu = cp.tile([128, 2, PL], f16, name="iucu", tag="iucu")
                    nc.vector.tensor_tensor(
                        out=sap(iucu[:], 0, [[PL, 2], [1, PL]]),
                        in0=sap(numic[:], 0, [[PL, 2], [1, PL]]),
                        in1=sap(rs16[:], 0, [[0, 2], [1, PL]]),
                        op=op.mult,
                    )
                    # move nn1 halves down to partitions 0-63 for the 2-view fusion
                    iucu2 = cp.tile([64, 2, PL], f16, name="iucu2", tag="iucu2")
                    nc.sync.dma_start(out=iucu2[:], in_=iucu[64:128])
                    lo = slice(0, 64)
                    # w1 = sigmoid(cu1-cu0) via tanh; 2*fused = (iu0+iu1) + t*(iu1-iu0)
                    dldi = cp.tile([64, 2, PL], f16, name="dldi", tag="dldi")
                    nc.vector.tensor_tensor(
                        out=dldi[:], in0=iucu2[:], in1=iucu[lo], op=op.subtract
                    )
                    tt = cp.tile([64, PL], f16, name="tt", tag="tt")
                    nc.scalar.activation(out=tt[:], in_=dldi[:, 1, :], func=Act.Tanh, scale=0.5)
                    sm = cp.tile([64, PL], f16, name="sm", tag="sm")
                    nc.vector.tensor_tensor(
                        out=sm[:], in0=iucu2[:, 0, :], in1=iucu[lo][:, 0, :], op=op.add
                    )
                    mg = cp.tile([64, PL], f16, name="mg", tag="mg")
                    nc.vector.tensor_tensor(out=mg[:], in0=tt[:], in1=dldi[:, 0, :], op=op.mult)
                    nc.vector.tensor_tensor(out=sm[:], in0=mg[:], in1=sm[:], op=op.add)
                    # a = g*0.5*(dmin-dmax) + dmax, written (w,q2)-interleaved so
                    # the final recip + output DMA are fully contiguous
                    a32 = cp.tile([64, PL], f32, name="a32", tag="a32")
                    nc.scalar.activation(
                        out=sap(a32[:], 0, [[1, UP], [UP, WC]]),
                        in_=sap(sm[:], 0, [[WC, UP], [1, WC]]),
                        func=Act.Identity,
                        scale=CC(C_DS2, 0, 64), bias=CC(C_DB, 0, 64),
                    )
                    tails.append((a32, gc, wc))
                    if ci >= 1:
                        emit_tail(tails[ci - 1])
                emit_tail(tails[-1])

    nc.finalize()
    return nc


def _host_prep(inputs):
    K_ref = np.asarray(inputs["K_ref"], np.float32)
    K_nei = np.asarray(inputs["K_nei"], np.float32)
    R_nei = np.asarray(inputs["R_nei"], np.float32)
    T_nei = np.asarray(inputs["T_nei"], np.float32)
    depth0 = np.asarray(inputs["depth0"], np.float32)
    flow = np.asarray(inputs["flow"], np.float32)
    mask = np.asarray(inputs["mask"], np.float32)
    conf = np.asarray(inputs["conf"], np.float32)
    dmin = float(np.asarray(inputs["depth_min"]).reshape(-1)[0])
    dmax = float(np.asarray(inputs["depth_max"]).reshape(-1)[0])

    # pixel rays per batch (u, v with unit z)
    uv = []
    for b in range(B):
        Ki = np.linalg.inv(K_ref[b, 0, 0].astype(np.float64))
        gx, gy = np.meshgrid(np.arange(W, dtype=np.float64), np.arange(H, dtype=np.float64))
        x = Ki[0, 0] * gx + Ki[0, 1] * gy + Ki[0, 2]
        y = Ki[1, 0] * gx + Ki[1, 1] * gy + Ki[1, 2]
        z = Ki[2, 0] * gx + Ki[2, 1] * gy + Ki[2, 2]
        uv.append((np.float32(x / z), np.float32(y / z)))

    cA = 1.0 / (dmin - dmax)
    cB = -dmax / (dmin - dmax)

    in_maps = []
    for c in range(NCORES):
        b, rc = c // 4, c % 4
        r0 = rc * RPC
        rtop = max(r0 - 1, 0)
        rbot = min(r0 + RPC, H - 1)

        consts = np.zeros((128, NCONST), np.float32)
        for nn in range(NN):
            Kn = K_nei[nn, b, 0, 0].astype(np.float64)
            Rn = R_nei[nn, b, 0, 0].astype(np.float64)
            Tn = T_nei[nn, b, 0, 0].astype(np.float64).reshape(3)
            M = Kn @ Rn
            t = (Kn @ Tn.reshape(3, 1)).reshape(3)
            iK = np.linalg.inv(Kn)
            assert abs(iK[2, 0]) < 1e-12 and abs(iK[2, 1]) < 1e-12 and abs(iK[2, 2] - 1) < 1e-9
            row = np.zeros(NCONST, np.float32)
            row[C_M00:C_M22 + 1] = M.reshape(-1)
            row[C_T0:C_T2 + 1] = t
            row[C_R00:C_R22 + 1] = Rn.reshape(-1)
            row[C_A0:C_A2 + 1] = iK[0] / (1.0 + EPS)
            row[C_B0:C_B2 + 1] = iK[1] / (1.0 + EPS)
            # C_TX/C_TY feed |tz*n + c| as ACT affine bias -> store negated
            row[C_TX], row[C_TY], row[C_TZ] = -Tn[0], -Tn[1], Tn[2]
            row[C_CA], row[C_CB] = cA, cB
            row[C_TEN] = 10.0
            row[C_DS2], row[C_DB] = 0.5 * (dmin - dmax), dmax
            consts[nn * 64 : nn * 64 + 64] = row

        u_full, v_full = uv[b]
        d_full = depth0[b, 0]

        pix = np.zeros((128, 4, 330), np.float32)
        for nn in range(NN):
            sl = slice(nn * 64, nn * 64 + 64)
            f_full = flow[nn, b, 0]
            for ch, arr in enumerate((u_full, v_full, d_full, f_full)):
                pix[sl, ch, 0:320] = arr[r0 : r0 + RPC]
                pix[sl, ch, 320:325] = arr[rtop].reshape(64, 5)
                pix[sl, ch, 325:330] = arr[rbot].reshape(64, 5)

        hm = np.ones((128, 10), np.float16)
        if r0 == 0:
            hm[:, 0:5] = 0.0
        if r0 + RPC == H:
            hm[:, 5:10] = 0.0

        confpad = np.zeros((NN, 66, 322), np.float16)
        confpad[:, 1:65, 1:321] = conf[:, b, 0, r0 : r0 + RPC, :]
        if r0 > 0:
            confpad[:, 0, 1:321] = conf[:, b, 0, r0 - 1, :]
        if r0 + RPC < H:
            confpad[:, 65, 1:321] = conf[:, b, 0, r0 + RPC, :]

        # [nn, k, q1, q2, r, wc, w] -> [q1, wc, (nn, r), k, q2, w]
        ms = mask[:, b, :, r0 : r0 + RPC, :].reshape(NN, 9, 4, 4, RPC, 2, WC)
        mask_pk = np.ascontiguousarray(ms.transpose(2, 5, 0, 4, 1, 3, 6)).reshape(
            4, 2, 128, KQ
        ).astype(np.float16)

        in_maps.append(
            {
                "pix": pix,
                "consts": consts,
                "hm": hm,
                "confpad": confpad,
                "maskpk": mask_pk,
            }
        )
    return in_maps


def kernel(**inputs):
    if "nc" not in _cache:
        _cache["nc"] = _build_program()
    nc = _cache["nc"]
    in_maps = _host_prep(inputs)

    from concourse import bass_utils

    res = bass_utils.run_bass_kernel_spmd(nc, in_maps, core_ids=list(range(NCORES)))
    out = np.empty((B, 1, H * UP, W * UP), np.float32)
    for c in range(NCORES):
        b, rc = c // 4, c % 4
        out[b, 0, rc * RPC * UP : (rc + 1) * RPC * UP, :] = res.results[c]["out"]
    return out
